# revision 27
# baseline (speedup 1.0000x reference)
"""MoE (gating + 8 experts, BN-folded) Trainium2 Bass kernel, v3.

Contract: kernel(**inputs) takes the FULL unsharded inputs (numpy, keyed as in
setup_inputs()) and returns the FULL [65536, 1] float32 output.

Strategy (v3):
  * Data-parallel over 8 NeuronCores: batch 65536 -> 8192 rows per core.
  * All BatchNorms folded into the adjacent Linear weights/biases on host.
  * Activations live as [features(partitions), batch(free)]; x is transposed
    host-side per shard and stays resident in SBUF (32 KB/partition).
  * Every matmul is a full-array instruction (K=128 chunks, M=128, N=512):
    small-M/small-K matmuls (gating L2, pair output projection) are padded to
    M=128 because col_grp-tiled matmuls expose ~190 ns of LDWEIGHTS +
    pipeline overhead each on TRN2, while padding is free (cost is N cycles).
  * Expert-pair-major: 4 phases, one per expert pair; per batch tile a block
    runs L0(a) L0(b) L1(a) L1(b) [pipelined: L2(pair,t-1), Zproj(pair,t-2)].
    The expert L2 (H1=128 -> H2=64 twice) uses block-diagonal [256->128]
    weights so the pair's outputs stack into one 128-partition tile.
  * Zproj multiplies h2 with [ow;0 | 0;ow] padded to M=128 -> z[2,512] per
    (pair, tile); z and the raw gate numerators exp(logits) are exported and
    the host computes y = sum_e g_e z_e / sum_e g_e + ob in float64.
  * Gating (2+1 matmuls per tile) is interleaved 4 tiles per pair phase so
    the PE never idles on the x DMA stream and the kernel has no tail phase.
"""

import numpy as np
import ml_dtypes

EPS = 1e-5
B, D, E, G, H0, H1, H2 = 65536, 256, 8, 128, 256, 128, 64
NCORES = 8
NB = B // NCORES          # rows per core
TB = 512                  # batch tile (matmul free dim / PSUM bank)
NT = NB // TB             # batch tiles per core
KD = D // 128             # k-chunks over D
NPAIR = E // 2


def _fold_params(inputs):
    """Fold the four BatchNorms into the adjacent Linears. float64 math."""
    f = {k: np.asarray(v, dtype=np.float64) for k, v in inputs.items()}

    s_in = f["in_g"] / np.sqrt(f["in_v"] + EPS)            # [D]
    t_in = f["in_b"] - f["in_m"] * s_in                    # [D]

    # gating L1 (+input BN folded in)
    a_g = f["g_g"] / np.sqrt(f["g_v"] + EPS)               # [G]
    w1 = f["gW1"] * a_g[None, :]                           # [D,G]
    W1f = s_in[:, None] * w1
    b1f = t_in @ w1 + (f["gb1"] - f["g_m"]) * a_g + f["g_b"]

    # expert L0 (+input BN)
    a0 = f["e0g"] / np.sqrt(f["e0v"] + EPS)                # [E,H0]
    w0 = f["eW0"] * a0[:, None, :]                         # [E,D,H0]
    W0f = s_in[None, :, None] * w0
    b0f = np.einsum("d,edo->eo", t_in, w0) + (f["eb0"] - f["e0m"]) * a0 + f["e0b"]

    a1 = f["e1g"] / np.sqrt(f["e1v"] + EPS)
    W1ef = f["eW1"] * a1[:, None, :]                       # [E,H0,H1]
    b1ef = (f["eb1"] - f["e1m"]) * a1 + f["e1b"]

    a2 = f["e2g"] / np.sqrt(f["e2v"] + EPS)
    W2f = f["eW2"] * a2[:, None, :]                        # [E,H1,H2]
    b2f = (f["eb2"] - f["e2m"]) * a2 + f["e2b"]

    g32 = lambda a: np.ascontiguousarray(a, dtype=np.float32)
    gbf = lambda a: np.ascontiguousarray(a, dtype=np.float32).astype(ml_dtypes.bfloat16)

    dev = {}
    dev["WG1"] = gbf(W1f.reshape(KD, 128, G).transpose(1, 0, 2))          # [128,KD,G]
    dev["BG1"] = g32(b1f[:, None])                                        # [G,1]
    WG2 = np.zeros((G, 128), dtype=np.float64)
    WG2[:, 0:E] = f["gW2"]                                                # M padded
    dev["WG2"] = gbf(WG2)                                                 # [G,128]
    dev["BG2"] = g32(f["gb2"][:, None])                                   # [E,1]
    dev["WE0"] = gbf(W0f.reshape(E, KD, 128, 2, 128).transpose(2, 0, 1, 3, 4))  # [128,E,KD,2,128]
    dev["BE0"] = g32(b0f.reshape(E, 2, 128).transpose(2, 0, 1))           # [128,E,2]
    dev["WE1"] = gbf(W1ef.reshape(E, 2, 128, H1).transpose(2, 0, 1, 3))   # [128,E,2,H1]
    dev["BE1"] = g32(b1ef.T)                                              # [H1,E]
    WE2 = np.zeros((128, NPAIR, 2, 128), dtype=np.float64)                # block-diag pair
    BE2 = np.zeros((128, NPAIR), dtype=np.float64)
    for j in range(NPAIR):
        WE2[:, j, 0, 0:64] = W2f[2 * j]
        WE2[:, j, 1, 64:128] = W2f[2 * j + 1]
        BE2[0:64, j] = b2f[2 * j]
        BE2[64:128, j] = b2f[2 * j + 1]
    dev["WE2"] = gbf(WE2)
    dev["BE2"] = g32(BE2)
    ow = f["oW"][:, 0]                                                    # [H2]
    OWPF = np.zeros((128, 128), dtype=np.float64)                         # M padded
    OWPF[0:64, 0] = ow
    OWPF[64:128, 1] = ow
    dev["OWPF"] = gbf(OWPF)
    ob = float(f["ob"][0])
    return dev, ob


def _build_program():
    import concourse.bass as bass
    import concourse.mybir as mybir
    import concourse.tile as tile
    from concourse import bacc

    f32 = mybir.dt.float32
    bf16 = mybir.dt.bfloat16
    Relu = mybir.ActivationFunctionType.Relu
    Exp = mybir.ActivationFunctionType.Exp
    Copy = mybir.ActivationFunctionType.Copy
    add = mybir.AluOpType.add
    amax = mybir.AluOpType.max

    nc = bacc.Bacc("TRN2", target_bir_lowering=False, debug=False)

    xT = nc.dram_tensor("xT", [D, NB], bf16, kind="ExternalInput").ap()
    zs = nc.dram_tensor("zs", [NPAIR, NT // 4, 2, 4, TB], f32, kind="ExternalOutput").ap()
    eg = nc.dram_tensor("eg", [NPAIR, E, 4, TB], f32, kind="ExternalOutput").ap()

    d_in = {}
    shapes = {
        "WG1": ([128, KD, G], bf16), "BG1": ([G, 1], f32),
        "WG2": ([G, 128], bf16), "BG2": ([E, 1], f32),
        "WE0": ([128, E, KD, 2, 128], bf16), "BE0": ([128, E, 2], f32),
        "WE1": ([128, E, 2, H1], bf16), "BE1": ([H1, E], f32),
        "WE2": ([128, NPAIR, 2, 128], bf16), "BE2": ([128, NPAIR], f32),
        "OWPF": ([128, 128], bf16),
    }
    for name, (shape, dt) in shapes.items():
        d_in[name] = nc.dram_tensor(name, shape, dt, kind="ExternalInput").ap()

    with tile.TileContext(nc) as tc:
        with (
            tc.tile_pool(name="consts", bufs=1) as consts,
            tc.tile_pool(name="xp", bufs=NT) as xp,
            tc.tile_pool(name="ghp", bufs=3) as ghp,
            tc.tile_pool(name="egp", bufs=2) as egp,
            tc.tile_pool(name="h0p", bufs=6) as h0p,
            tc.tile_pool(name="h1p", bufs=8) as h1p,
            tc.tile_pool(name="h2p", bufs=6) as h2p,
            tc.tile_pool(name="zbp", bufs=3) as zbp,
            tc.tile_pool(name="p0", bufs=2, space="PSUM") as p0,     # 2x [128,2,512] = 4 banks
            tc.tile_pool(name="p1", bufs=2, space="PSUM") as p1,     # 2x [128,512]   = 2 banks
            tc.tile_pool(name="p2", bufs=1, space="PSUM") as p2,     # 1x [128,512]   = 1 bank
            tc.tile_pool(name="pz", bufs=1, space="PSUM") as pz,     # 1x [128,512]   = 1 bank
        ):
            W = {}
            for name, ap in d_in.items():
                W[name] = consts.tile(list(ap.shape), shapes[name][1], tag=name,
                                      name=name)
            # Expert weights stream on the gpsimd queue in usage order; the
            # small biases ride the vector queue and the gating consts the
            # scalar queue so nothing serializes behind the big transfers.
            def load_pair(j):
                for i in (0, 1):
                    e = 2 * j + i
                    nc.gpsimd.dma_start(W["WE0"][:, e], d_in["WE0"][:, e])
                    nc.gpsimd.dma_start(W["WE1"][:, e], d_in["WE1"][:, e])
                nc.gpsimd.dma_start(W["WE2"][:, j], d_in["WE2"][:, j])

            load_pair(0)
            for name in ("BE0", "BE1", "WG1", "BG1", "BE2", "WG2", "BG2", "OWPF"):
                nc.scalar.dma_start(W[name][:], d_in[name][:])

            # first 4 x tiles up-front; the rest stream during phase 0 so the
            # early DMA bandwidth goes to the pair-0 weights.
            xt = []

            def load_x(t):
                bs = t * TB
                xti = xp.tile([128, KD, TB], bf16, tag="xt", name=f"x{t}")
                for c in range(KD):
                    nc.sync.dma_start(xti[:, c, :], xT[c * 128:(c + 1) * 128, bs:bs + TB])
                xt.append(xti)

            for t in range(4):
                load_x(t)

            def gating_a(t):
                ps_g = p0.tile([128, 2, TB], f32, tag="mm0", name="ps_g")
                for c in range(KD):
                    nc.tensor.matmul(ps_g[:, 0, :], W["WG1"][:, c, :], xt[t][:, c, :],
                                     start=(c == 0), stop=(c == KD - 1))
                gh = ghp.tile([128, TB], bf16, tag="gh")
                nc.scalar.activation(gh[:], ps_g[:, 0, :], Relu,
                                     bias=W["BG1"][:, 0:1])
                return gh

            def gating_b(gh, egb, q):
                # exports raw logits (+bias); the host applies exp
                ps_l = p1.tile([128, TB], f32, tag="mm1", name="ps_l")
                nc.tensor.matmul(ps_l[:], W["WG2"][:], gh[:], start=True, stop=True)
                nc.vector.tensor_scalar_add(egb[0:E, q, :], ps_l[0:E, :],
                                            W["BG2"][:, 0:1])

            # PE warm-up: the HAM clock gate needs ~3.4us of sustained matmul
            # activity to lift the PE from 1.2 to 2.4 GHz; burn the DMA fill
            # time on dummy matmuls over a zeroed tile so the real matmuls
            # start warm.
            warm = consts.tile([128, TB], bf16, tag="warm", name="warm")
            nc.vector.memset(warm[:], 0.0)
            psw = pz.tile([128, TB], f32, tag="z", name="psw")
            for _ in range(6):
                nc.tensor.matmul(psw[:], warm[:, 0:128], warm[:],
                                 start=True, stop=True)

            # ---- expert pair phases; gating interleaved 4 tiles per phase ----
            for j in range(NPAIR):
                ea, eb = 2 * j, 2 * j + 1
                h1t = {}
                h2t = {}
                egb = None
                zb4 = None
                gh = None
                for t in range(NT + 3):
                    if j == 0 and 1 <= t <= 12:
                        load_x(t + 3)
                        if t % 4 == 0:
                            load_pair(t // 4)

                    if t < NT and t % 4 == 1:
                        # gating L1 a block early so gh has a full block of slack
                        gh = gating_a(4 * j + (t + 1) // 4)
                    if t < NT and t % 4 == 2:
                        if t == 2:
                            egb = egp.tile([E, 4, TB], f32, tag="eg", name="egb")
                        gating_b(gh, egb, t // 4)
                    if t < NT:
                        # L0 for both experts of the pair
                        ps0 = [p0.tile([128, 2, TB], f32, tag="mm0", name=f"ps0_{i}")
                               for i in (0, 1)]
                        h0 = [h0p.tile([128, 2, TB], bf16, tag="h0", name=f"h0_{i}")
                              for i in (0, 1)]
                        for i, ex in ((0, ea), (1, eb)):
                            for mc in range(2):
                                for c in range(KD):
                                    nc.tensor.matmul(ps0[i][:, mc, :],
                                                     W["WE0"][:, ex, c, mc, :],
                                                     xt[t][:, c, :],
                                                     start=(c == 0), stop=(c == KD - 1))
                            nc.scalar.activation(h0[i][:, 0, :], ps0[i][:, 0, :], Relu,
                                                 bias=W["BE0"][:, ex, 0:1])
                            nc.vector.tensor_scalar(out=h0[i][:, 1, :], in0=ps0[i][:, 1, :],
                                                    scalar1=W["BE0"][:, ex, 1:2],
                                                    scalar2=0.0, op0=add, op1=amax)
                        # L1 for both experts
                        ps1 = [p1.tile([128, TB], f32, tag="mm1", name=f"ps1_{i}")
                               for i in (0, 1)]
                        h1 = [h1p.tile([128, TB], bf16, tag="h1", name=f"h1_{i}")
                              for i in (0, 1)]
                        for i, ex in ((0, ea), (1, eb)):
                            for c in range(2):
                                nc.tensor.matmul(ps1[i][:], W["WE1"][:, ex, c, :],
                                                 h0[i][:, c, :],
                                                 start=(c == 0), stop=(c == 1))
                        nc.scalar.activation(h1[0][:], ps1[0][:], Relu,
                                             bias=W["BE1"][:, ea:ea + 1])
                        nc.vector.tensor_scalar(out=h1[1][:], in0=ps1[1][:],
                                                scalar1=W["BE1"][:, eb:eb + 1],
                                                scalar2=0.0, op0=add, op1=amax)
                        h1t[t] = h1
                        if t == 14:
                            nc.sync.dma_start(eg[j], egb[:])
                    if 2 <= t < NT + 2:
                        # L2 for tile t-2: block-diagonal pair weights
                        tm = t - 2
                        ps2 = p2.tile([128, TB], f32, tag="mm2", name="ps2")
                        for c in range(2):
                            nc.tensor.matmul(ps2[:], W["WE2"][:, j, c, :],
                                             h1t[tm][c][:], start=(c == 0), stop=(c == 1))
                        h2 = h2p.tile([128, TB], bf16, tag="h2")
                        nc.scalar.activation(h2[:], ps2[:], Relu,
                                             bias=W["BE2"][:, j:j + 1])
                        h2t[tm] = h2
                        del h1t[tm]
                    if t >= 3:
                        # Zproj for tile t-3 (M padded to 128; rows 0:2 useful)
                        tz = t - 3
                        psz = pz.tile([128, TB], f32, tag="z", name="psz")
                        nc.tensor.matmul(psz[:], W["OWPF"][:], h2t[tz][:],
                                         start=True, stop=True)
                        del h2t[tz]
                        if tz % 4 == 0:
                            zb4 = zbp.tile([2, 4, TB], f32, tag="zb", name="zb4")
                        nc.vector.tensor_scalar_add(zb4[:, tz % 4, :], psz[0:2, :], 0.0)
                        if tz % 4 == 1:
                            nc.sync.dma_start(zs[j, tz // 4, :, 0:2], zb4[:, 0:2, :])
                        elif tz % 4 == 3:
                            nc.sync.dma_start(zs[j, tz // 4, :, 2:4], zb4[:, 2:4, :])

    nc.compile()
    return nc


_CACHE = {}


def _get_program():
    if "nc" not in _CACHE:
        _CACHE["nc"] = _build_program()
    return _CACHE["nc"]


def _run(inputs, trace=False):
    from concourse.bass_utils import run_bass_kernel_spmd

    x = np.ascontiguousarray(np.asarray(inputs["x"], dtype=np.float32))
    dev, ob = _fold_params(inputs)
    nc = _get_program()

    in_maps = []
    for c in range(NCORES):
        m = dict(dev)
        xs = np.ascontiguousarray(x[c * NB:(c + 1) * NB, :].T)
        m["xT"] = xs.astype(ml_dtypes.bfloat16)
        in_maps.append(m)

    kwargs = {}
    if trace:
        kwargs = dict(trace=True, trace_cores=[0])
    res = run_bass_kernel_spmd(nc, in_maps, core_ids=list(range(NCORES)), **kwargs)

    outs = []
    for c in range(NCORES):
        z = res.results[c]["zs"].astype(np.float64)      # [NPAIR, NT//4, 2, 4, TB]
        g = np.exp(res.results[c]["eg"].astype(np.float64))  # [NPAIR, E, 4, TB] logits
        # z[j, g, i, q, col] -> [t=4g+q, e=2j+i, col]
        z2 = z.transpose(1, 3, 0, 2, 4).reshape(NT, E, TB)
        # g[j, e, q, col]: gating tile t=4j+q -> [t, e, col]
        g2 = g.transpose(0, 2, 1, 3).reshape(NT, E, TB)
        num = np.sum(g2 * z2, axis=1)                    # [NT, TB]
        den = np.sum(g2, axis=1)
        outs.append((num / den).reshape(-1))
    out = np.concatenate(outs) + ob
    return out.astype(np.float32)[:, None], res


def kernel(**inputs):
    out, _ = _run(inputs, trace=False)
    return out


def kernel_traced(**inputs):
    return _run(inputs, trace=True)


# revision 29
# speedup vs baseline: 1.1742x; 1.1742x over previous
"""MoE (gating + 8 experts, BN-folded) Trainium2 Bass kernel, v3.

Contract: kernel(**inputs) takes the FULL unsharded inputs (numpy, keyed as in
setup_inputs()) and returns the FULL [65536, 1] float32 output.

Strategy (v3):
  * Data-parallel over 8 NeuronCores: batch 65536 -> 8192 rows per core.
  * All BatchNorms folded into the adjacent Linear weights/biases on host.
  * Activations live as [features(partitions), batch(free)]; x is transposed
    host-side per shard and stays resident in SBUF (32 KB/partition).
  * Every matmul is a full-array instruction (K=128 chunks, M=128, N=512):
    small-M/small-K matmuls (gating L2, pair output projection) are padded to
    M=128 because col_grp-tiled matmuls expose ~190 ns of LDWEIGHTS +
    pipeline overhead each on TRN2, while padding is free (cost is N cycles).
  * Expert-pair-major: 4 phases, one per expert pair; per batch tile a block
    runs L0(a) L0(b) L1(a) L1(b) [pipelined: L2(pair,t-1), Zproj(pair,t-2)].
    The expert L2 (H1=128 -> H2=64 twice) uses block-diagonal [256->128]
    weights so the pair's outputs stack into one 128-partition tile.
  * Zproj multiplies h2 with [ow;0 | 0;ow] padded to M=128 -> z[2,512] per
    (pair, tile); z and the raw gate numerators exp(logits) are exported and
    the host computes y = sum_e g_e z_e / sum_e g_e + ob in float64.
  * Gating (2+1 matmuls per tile) is interleaved 4 tiles per pair phase so
    the PE never idles on the x DMA stream and the kernel has no tail phase.
"""

import numpy as np
import ml_dtypes

EPS = 1e-5
B, D, E, G, H0, H1, H2 = 65536, 256, 8, 128, 256, 128, 64
NCORES = 8
NB = B // NCORES          # rows per core
TB = 512                  # batch tile (matmul free dim / PSUM bank)
NT = NB // TB             # batch tiles per core
KD = D // 128             # k-chunks over D
NPAIR = E // 2


def _fold_params(inputs):
    """Fold the four BatchNorms into the adjacent Linears. float64 math."""
    f = {k: np.asarray(v, dtype=np.float64) for k, v in inputs.items()}

    s_in = f["in_g"] / np.sqrt(f["in_v"] + EPS)            # [D]
    t_in = f["in_b"] - f["in_m"] * s_in                    # [D]

    # gating L1 (+input BN folded in)
    a_g = f["g_g"] / np.sqrt(f["g_v"] + EPS)               # [G]
    w1 = f["gW1"] * a_g[None, :]                           # [D,G]
    W1f = s_in[:, None] * w1
    b1f = t_in @ w1 + (f["gb1"] - f["g_m"]) * a_g + f["g_b"]

    # expert L0 (+input BN)
    a0 = f["e0g"] / np.sqrt(f["e0v"] + EPS)                # [E,H0]
    w0 = f["eW0"] * a0[:, None, :]                         # [E,D,H0]
    W0f = s_in[None, :, None] * w0
    b0f = np.einsum("d,edo->eo", t_in, w0) + (f["eb0"] - f["e0m"]) * a0 + f["e0b"]

    a1 = f["e1g"] / np.sqrt(f["e1v"] + EPS)
    W1ef = f["eW1"] * a1[:, None, :]                       # [E,H0,H1]
    b1ef = (f["eb1"] - f["e1m"]) * a1 + f["e1b"]

    a2 = f["e2g"] / np.sqrt(f["e2v"] + EPS)
    W2f = f["eW2"] * a2[:, None, :]                        # [E,H1,H2]
    b2f = (f["eb2"] - f["e2m"]) * a2 + f["e2b"]

    g32 = lambda a: np.ascontiguousarray(a, dtype=np.float32)
    gbf = lambda a: np.ascontiguousarray(a, dtype=np.float32).astype(ml_dtypes.bfloat16)

    dev = {}
    dev["WG1"] = gbf(W1f.reshape(KD, 128, G).transpose(1, 0, 2))          # [128,KD,G]
    dev["BG1"] = g32(b1f[:, None])                                        # [G,1]
    WG2 = np.zeros((G, 128), dtype=np.float64)
    WG2[:, 0:E] = f["gW2"]                                                # M padded
    dev["WG2"] = gbf(WG2)                                                 # [G,128]
    dev["BG2"] = g32(f["gb2"][:, None])                                   # [E,1]
    dev["WE0"] = gbf(W0f.reshape(E, KD, 128, 2, 128).transpose(2, 0, 1, 3, 4))  # [128,E,KD,2,128]
    dev["BE0"] = g32(b0f.reshape(E, 2, 128).transpose(2, 0, 1))           # [128,E,2]
    dev["WE1"] = gbf(W1ef.reshape(E, 2, 128, H1).transpose(2, 0, 1, 3))   # [128,E,2,H1]
    dev["BE1"] = g32(b1ef.T)                                              # [H1,E]
    WE2 = np.zeros((128, NPAIR, 2, 128), dtype=np.float64)                # block-diag pair
    BE2 = np.zeros((128, NPAIR), dtype=np.float64)
    for j in range(NPAIR):
        WE2[:, j, 0, 0:64] = W2f[2 * j]
        WE2[:, j, 1, 64:128] = W2f[2 * j + 1]
        BE2[0:64, j] = b2f[2 * j]
        BE2[64:128, j] = b2f[2 * j + 1]
    dev["WE2"] = gbf(WE2)
    dev["BE2"] = g32(BE2)
    ow = f["oW"][:, 0]                                                    # [H2]
    OWPF = np.zeros((128, 128), dtype=np.float64)                         # M padded
    OWPF[0:64, 0] = ow
    OWPF[64:128, 1] = ow
    dev["OWPF"] = gbf(OWPF)
    ob = float(f["ob"][0])
    return dev, ob


def _build_program():
    import concourse.bass as bass
    import concourse.mybir as mybir
    import concourse.tile as tile
    from concourse import bacc

    f32 = mybir.dt.float32
    bf16 = mybir.dt.bfloat16
    Relu = mybir.ActivationFunctionType.Relu
    Exp = mybir.ActivationFunctionType.Exp
    Copy = mybir.ActivationFunctionType.Copy
    add = mybir.AluOpType.add
    amax = mybir.AluOpType.max

    nc = bacc.Bacc("TRN2", target_bir_lowering=False, debug=False)

    xT = nc.dram_tensor("xT", [D, NB], bf16, kind="ExternalInput").ap()
    zs = nc.dram_tensor("zs", [NPAIR, NT // 4, 2, 4, TB], f32, kind="ExternalOutput").ap()
    eg = nc.dram_tensor("eg", [NPAIR, E, 4, TB], f32, kind="ExternalOutput").ap()

    d_in = {}
    shapes = {
        "WG1": ([128, KD, G], bf16), "BG1": ([G, 1], f32),
        "WG2": ([G, 128], bf16), "BG2": ([E, 1], f32),
        "WE0": ([128, E, KD, 2, 128], bf16), "BE0": ([128, E, 2], f32),
        "WE1": ([128, E, 2, H1], bf16), "BE1": ([H1, E], f32),
        "WE2": ([128, NPAIR, 2, 128], bf16), "BE2": ([128, NPAIR], f32),
        "OWPF": ([128, 128], bf16),
    }
    for name, (shape, dt) in shapes.items():
        d_in[name] = nc.dram_tensor(name, shape, dt, kind="ExternalInput").ap()

    with tile.TileContext(nc) as tc:
        with (
            tc.tile_pool(name="consts", bufs=1) as consts,
            tc.tile_pool(name="xp", bufs=NT) as xp,
            tc.tile_pool(name="ghp", bufs=3) as ghp,
            tc.tile_pool(name="egp", bufs=2) as egp,
            tc.tile_pool(name="h0p", bufs=6) as h0p,
            tc.tile_pool(name="h1p", bufs=8) as h1p,
            tc.tile_pool(name="h2p", bufs=6) as h2p,
            tc.tile_pool(name="zbp", bufs=3) as zbp,
            tc.tile_pool(name="p0", bufs=2, space="PSUM") as p0,     # 2x [128,2,512] = 4 banks
            tc.tile_pool(name="p1", bufs=2, space="PSUM") as p1,     # 2x [128,512]   = 2 banks
            tc.tile_pool(name="p2", bufs=1, space="PSUM") as p2,     # 1x [128,512]   = 1 bank
            tc.tile_pool(name="pz", bufs=1, space="PSUM") as pz,     # 1x [128,512]   = 1 bank
        ):
            W = {}
            for name, ap in d_in.items():
                W[name] = consts.tile(list(ap.shape), shapes[name][1], tag=name,
                                      name=name)
            # Expert weights stream on the gpsimd queue in usage order; the
            # small biases ride the vector queue and the gating consts the
            # scalar queue so nothing serializes behind the big transfers.
            def load_pair(j):
                for i in (0, 1):
                    e = 2 * j + i
                    nc.gpsimd.dma_start(W["WE0"][:, e], d_in["WE0"][:, e])
                    nc.gpsimd.dma_start(W["WE1"][:, e], d_in["WE1"][:, e])
                nc.gpsimd.dma_start(W["WE2"][:, j], d_in["WE2"][:, j])

            load_pair(0)
            for name in ("BE0", "BE1", "WG1", "BG1", "BE2", "WG2", "BG2", "OWPF"):
                nc.scalar.dma_start(W[name][:], d_in[name][:])

            # first 4 x tiles up-front; the rest stream during phase 0 so the
            # early DMA bandwidth goes to the pair-0 weights.
            xt = []

            def load_x(t):
                bs = t * TB
                xti = xp.tile([128, KD, TB], bf16, tag="xt", name=f"x{t}")
                for c in range(KD):
                    nc.sync.dma_start(xti[:, c, :], xT[c * 128:(c + 1) * 128, bs:bs + TB])
                xt.append(xti)

            for t in range(4):
                load_x(t)

            def gating_a(t):
                ps_g = p0.tile([128, 2, TB], f32, tag="mm0", name="ps_g")
                for c in range(KD):
                    nc.tensor.matmul(ps_g[:, 0, :], W["WG1"][:, c, :], xt[t][:, c, :],
                                     start=(c == 0), stop=(c == KD - 1))
                gh = ghp.tile([128, TB], bf16, tag="gh")
                nc.vector.tensor_scalar(out=gh[:], in0=ps_g[:, 0, :],
                                        scalar1=W["BG1"][:, 0:1], scalar2=0.0,
                                        op0=add, op1=amax)
                return gh

            def gating_b(gh, egb, q):
                ps_l = p1.tile([128, TB], f32, tag="mm1", name="ps_l")
                nc.tensor.matmul(ps_l[:], W["WG2"][:], gh[:], start=True, stop=True)
                nc.scalar.activation(egb[0:E, q, :], ps_l[0:E, :], Exp,
                                     bias=W["BG2"][:, 0:1])

            # PE warm-up: the HAM clock gate needs ~3.4us of sustained matmul
            # activity to lift the PE from 1.2 to 2.4 GHz; burn the DMA fill
            # time on dummy matmuls over a zeroed tile so the real matmuls
            # start warm.
            warm = consts.tile([128, TB], bf16, tag="warm", name="warm")
            nc.vector.memset(warm[:], 0.0)
            psw = pz.tile([128, TB], f32, tag="z", name="psw")
            for _ in range(6):
                nc.tensor.matmul(psw[:], warm[:, 0:128], warm[:],
                                 start=True, stop=True)

            # ---- expert pair phases; gating interleaved 4 tiles per phase ----
            for j in range(NPAIR):
                ea, eb = 2 * j, 2 * j + 1
                h1t = {}
                h2t = {}
                egb = None
                zb4 = None
                gh = None
                for t in range(NT + 3):
                    if j == 0 and 1 <= t <= 12:
                        load_x(t + 3)
                        if t % 4 == 0:
                            load_pair(t // 4)

                    if t < NT and t % 4 == 1:
                        # gating L1 a block early so gh has a full block of slack
                        gh = gating_a(4 * j + (t + 1) // 4)
                    if t < NT and t % 4 == 2:
                        if t == 2:
                            egb = egp.tile([E, 4, TB], f32, tag="eg", name="egb")
                        gating_b(gh, egb, t // 4)
                    if t < NT:
                        # L0 for both experts of the pair
                        ps0 = [p0.tile([128, 2, TB], f32, tag="mm0", name=f"ps0_{i}")
                               for i in (0, 1)]
                        h0 = [h0p.tile([128, 2, TB], bf16, tag="h0", name=f"h0_{i}")
                              for i in (0, 1)]
                        for i, ex in ((0, ea), (1, eb)):
                            for mc in range(2):
                                for c in range(KD):
                                    nc.tensor.matmul(ps0[i][:, mc, :],
                                                     W["WE0"][:, ex, c, mc, :],
                                                     xt[t][:, c, :],
                                                     start=(c == 0), stop=(c == KD - 1))
                            nc.scalar.activation(h0[i][:, 0, :], ps0[i][:, 0, :], Relu,
                                                 bias=W["BE0"][:, ex, 0:1])
                            nc.vector.tensor_scalar(out=h0[i][:, 1, :], in0=ps0[i][:, 1, :],
                                                    scalar1=W["BE0"][:, ex, 1:2],
                                                    scalar2=0.0, op0=add, op1=amax)
                        # L1 for both experts
                        ps1 = [p1.tile([128, TB], f32, tag="mm1", name=f"ps1_{i}")
                               for i in (0, 1)]
                        h1 = [h1p.tile([128, TB], bf16, tag="h1", name=f"h1_{i}")
                              for i in (0, 1)]
                        for i, ex in ((0, ea), (1, eb)):
                            for c in range(2):
                                nc.tensor.matmul(ps1[i][:], W["WE1"][:, ex, c, :],
                                                 h0[i][:, c, :],
                                                 start=(c == 0), stop=(c == 1))
                        nc.scalar.activation(h1[0][:], ps1[0][:], Relu,
                                             bias=W["BE1"][:, ea:ea + 1])
                        nc.vector.tensor_scalar(out=h1[1][:], in0=ps1[1][:],
                                                scalar1=W["BE1"][:, eb:eb + 1],
                                                scalar2=0.0, op0=add, op1=amax)
                        h1t[t] = h1
                        if t == 14:
                            nc.sync.dma_start(eg[j], egb[:])
                    if 2 <= t < NT + 2:
                        # L2 for tile t-2: block-diagonal pair weights
                        tm = t - 2
                        ps2 = p2.tile([128, TB], f32, tag="mm2", name="ps2")
                        for c in range(2):
                            nc.tensor.matmul(ps2[:], W["WE2"][:, j, c, :],
                                             h1t[tm][c][:], start=(c == 0), stop=(c == 1))
                        h2 = h2p.tile([128, TB], bf16, tag="h2")
                        nc.scalar.activation(h2[:], ps2[:], Relu,
                                             bias=W["BE2"][:, j:j + 1])
                        h2t[tm] = h2
                        del h1t[tm]
                    if t >= 3:
                        # Zproj for tile t-3 (M padded to 128; rows 0:2 useful)
                        tz = t - 3
                        psz = pz.tile([128, TB], f32, tag="z", name="psz")
                        nc.tensor.matmul(psz[:], W["OWPF"][:], h2t[tz][:],
                                         start=True, stop=True)
                        del h2t[tz]
                        if tz % 4 == 0:
                            zb4 = zbp.tile([2, 4, TB], f32, tag="zb", name="zb4")
                        nc.vector.tensor_scalar_add(zb4[:, tz % 4, :], psz[0:2, :], 0.0)
                        if tz % 4 == 1:
                            nc.sync.dma_start(zs[j, tz // 4, :, 0:2], zb4[:, 0:2, :])
                        elif tz % 4 == 3:
                            nc.sync.dma_start(zs[j, tz // 4, :, 2:4], zb4[:, 2:4, :])

    nc.compile()
    return nc


_CACHE = {}


def _get_program():
    if "nc" not in _CACHE:
        _CACHE["nc"] = _build_program()
    return _CACHE["nc"]


def _run(inputs, trace=False):
    from concourse.bass_utils import run_bass_kernel_spmd

    x = np.ascontiguousarray(np.asarray(inputs["x"], dtype=np.float32))
    dev, ob = _fold_params(inputs)
    nc = _get_program()

    in_maps = []
    for c in range(NCORES):
        m = dict(dev)
        xs = np.ascontiguousarray(x[c * NB:(c + 1) * NB, :].T)
        m["xT"] = xs.astype(ml_dtypes.bfloat16)
        in_maps.append(m)

    kwargs = {}
    if trace:
        kwargs = dict(trace=True, trace_cores=[0])
    res = run_bass_kernel_spmd(nc, in_maps, core_ids=list(range(NCORES)), **kwargs)

    outs = []
    for c in range(NCORES):
        z = res.results[c]["zs"].astype(np.float64)      # [NPAIR, NT//4, 2, 4, TB]
        g = res.results[c]["eg"].astype(np.float64)      # [NPAIR, E, 4, TB]
        # z[j, g, i, q, col] -> [t=4g+q, e=2j+i, col]
        z2 = z.transpose(1, 3, 0, 2, 4).reshape(NT, E, TB)
        # g[j, e, q, col]: gating tile t=4j+q -> [t, e, col]
        g2 = g.transpose(0, 2, 1, 3).reshape(NT, E, TB)
        num = np.sum(g2 * z2, axis=1)                    # [NT, TB]
        den = np.sum(g2, axis=1)
        outs.append((num / den).reshape(-1))
    out = np.concatenate(outs) + ob
    return out.astype(np.float32)[:, None], res


def kernel(**inputs):
    out, _ = _run(inputs, trace=False)
    return out


def kernel_traced(**inputs):
    return _run(inputs, trace=True)
